# revision 30
# baseline (speedup 1.0000x reference)
"""AlphaWeightedConv2d Trainium2 kernel.

Reference computation (B=32, CIN=COUT=64, H=W=112, K=3, pad=1):
    g = sigmoid(alpha[label])                     # [B, COUT]
    y = conv2d(x, W) * g[:,:,None,None] + (bias * g)[:,:,None,None]

Strategy: data-parallel over batch across 8 NeuronCores (4 samples/core).
Per core the conv is expressed as 9 shifted K=64 matmuls per output chunk
(CIN on partitions) over a host-padded image layout ([114,114] with zero
borders), so every conv tap is a plain (slot, column) window into one
contiguous SBUF tile and every DMA load is a single contiguous span per
partition (fat descriptors -> near line-rate HBM reads).  Two samples
ride in the two 64-partition halves of each tile; even/odd output chunks
map onto the four 64x64 quadrants of the PE array (4 concurrent matmul
streams, separate PSUM banks).  The sigmoid gate is computed on host
([32,64] - negligible) and applied by the DVE/ACT epilogue as a
per-partition scale+bias while evacuating PSUM.  Warm-up matmuls on a
memset tile release the PE HAM clock gate (1.2 -> 2.4 GHz) while the
first loads are in flight.  x is cast to bf16 on host (tolerance allows
it; halves input HBM traffic); output is written bf16 and upcast to f32
on host.
"""

import numpy as np
import ml_dtypes

B, CIN, COUT, H, W_SP = 32, 64, 64, 112, 112
N_CORES = 8
B_LOC = B // N_CORES          # 4 samples per core
SLOT = 114                    # padded row width (1 + 112 + 1)
NSLOT = 30                    # row slots per tile (28 rows + halo pair)
TW = NSLOT * SLOT             # 3420 tile width
R = 28                        # image rows per tile
NT = 4                        # row tiles per sample (4*28 = 112)
NCHUNK = (H // (NT * 4))      # 7 chunks per tile
TAPS = [(dy, dx) for dy in range(3) for dx in range(3)]

_cached = None


def _build():
    from concourse import bacc, tile, mybir

    bf16 = mybir.dt.bfloat16
    f32 = mybir.dt.float32
    mult = mybir.AluOpType.mult
    add = mybir.AluOpType.add
    ident = mybir.ActivationFunctionType.Identity

    nc = bacc.Bacc("TRN2", target_bir_lowering=False, debug=False,
                   num_devices=N_CORES)
    x_ext = nc.dram_tensor("x", [B_LOC * CIN, H + 2, SLOT], bf16,
                           kind="ExternalInput")
    w_ext = nc.dram_tensor("w", [128, 9 * 64], bf16, kind="ExternalInput")
    gs_ext = nc.dram_tensor("gs", [128, 4], f32, kind="ExternalInput")
    gb_ext = nc.dram_tensor("gb", [128, 4], f32, kind="ExternalInput")
    out_ext = nc.dram_tensor("out", [B_LOC * COUT, H, W_SP], bf16,
                             kind="ExternalOutput")

    with tile.TileContext(nc) as tc:
        with (
            tc.tile_pool(name="wpool", bufs=1) as wpool,
            tc.tile_pool(name="xpool", bufs=8) as xpool,
            tc.tile_pool(name="opool", bufs=8) as opool,
            tc.tile_pool(name="pspool", bufs=8, space="PSUM") as pspool,
        ):
            w = wpool.tile([128, 9 * 64], bf16)
            gs = wpool.tile([128, 4], f32)
            gb = wpool.tile([128, 4], f32)

            def load_tile(p, t, split=False):
                # tile t holds padded rows [28t, 28t+30): one contiguous
                # 6840B span per partition -> a single fat DMA descriptor
                xt = xpool.tile([128, TW], bf16, tag="xtile", name=f"x{p}{t}")
                dv = xt[:, :].rearrange("p (s j) -> p s j", j=SLOT)
                sv = x_ext.ap()[p * 128:(p + 1) * 128,
                                R * t:R * t + NSLOT, :]
                if split:
                    # first tile in k-iteration-sized pieces: the early
                    # stream is load-rate-limited (per-engine SDMA cap),
                    # so finer splits only starve later iterations
                    nc.sync.dma_start(dv[:, 0:10], sv[:, 0:10])
                    nc.sync.dma_start(dv[:, 10:20], sv[:, 10:20])
                    nc.sync.dma_start(dv[:, 20:NSLOT], sv[:, 20:NSLOT])
                else:
                    nc.sync.dma_start(dv, sv)
                return xt

            # critical chain first on the sync queue (the scalar queue's
            # HWDGE sits behind a ~1.3us ACT_TABLE_LOAD at startup)
            nc.sync.dma_start(w[:], w_ext.ap()[:])
            T_next = [load_tile(0, 0, split=True)]
            nc.scalar.dma_start(gs[:], gs_ext.ap()[:])
            nc.scalar.dma_start(gb[:], gb_ext.ap()[:])
            T_next += [load_tile(0, t) for t in range(1, NT)]

            # ---- PE warm-up: the HAM clock gate starts at 1.2 GHz and
            # needs ~3.4us of sustained matmul activity to release to
            # 2.4 GHz.  The PE is idle while the first loads are in
            # flight anyway, so burn that window on matmuls over a
            # memset tile (no DMA dependency - starts right after the
            # framework preamble). ----
            wu = wpool.tile([128, 512], bf16)
            nc.gpsimd.memset(wu[:], 0.0)
            psw = pspool.tile([128, 4 * W_SP], f32, tag="ps")
            for _ in range(10):
                nc.tensor.matmul(psw[:, :], wu[:, 0:128], wu[:, 0:448],
                                 start=True, stop=True)

            for p in range(2):  # sample pairs (2p, 2p+1)
                T = T_next
                T_next = []
                OSB = [opool.tile([128, R * W_SP], bf16, tag="osb",
                                  name=f"osb{p}{t}")
                       for t in range(NT)]

                # ---- 28 chunks in pairs: even->psE quadrants (0,0)/(64,64),
                #      odd->psO quadrants (0,64)/(64,0) ----
                for k in range(NT * NCHUNK // 2):
                    # spread the next pair's tile loads across this pair's
                    # compute, ahead of this k's flushes in queue order
                    if p == 0 and k in (1, 4, 7, 10):
                        T_next.append(load_tile(1, len(T_next)))
                    c0, c1 = 2 * k, 2 * k + 1
                    last = (p == 1 and k == NT * NCHUNK // 2 - 1)
                    if last:
                        # scheduler breaks ties by PSUM allocation order;
                        # the odd chunk gates the final flush, so give it
                        # priority in the tail
                        psO = pspool.tile([128, 4 * W_SP], f32, tag="ps",
                                          name="psO_l")
                        psE = pspool.tile([128, 4 * W_SP], f32, tag="ps",
                                          name="psE_l")
                    else:
                        psE = pspool.tile([128, 4 * W_SP], f32, tag="ps")
                        psO = pspool.tile([128, 4 * W_SP], f32, tag="ps")
                    for i, (dy, dx) in enumerate(TAPS):
                        st, sp = i == 0, i == 8
                        for c, ps, swap in ((c0, psE, False), (c1, psO, True)):
                            t, u = divmod(c, NCHUNK)
                            rv = T[t][:, :].rearrange(
                                "p (s j) -> p s j", j=SLOT)
                            ra = rv[0:64, 4 * u + dy:4 * u + dy + 4,
                                    dx:dx + 112]
                            rb = rv[64:128, 4 * u + dy:4 * u + dy + 4,
                                    dx:dx + 112]
                            aslice = ps[64:128] if swap else ps[0:64]
                            bslice = ps[0:64] if swap else ps[64:128]
                            nc.tensor.matmul(
                                aslice.rearrange("p (r j) -> p r j", j=W_SP),
                                w[0:64, i * 64:(i + 1) * 64],
                                ra, start=st, stop=sp)
                            nc.tensor.matmul(
                                bslice.rearrange("p (r j) -> p r j", j=W_SP),
                                w[64:128, i * 64:(i + 1) * 64],
                                rb, start=st, stop=sp)
                    def flush(t, ra_, rb_, eng):
                        dst = out_ext.ap()[p * 128:(p + 1) * 128,
                                           R * t + ra_:R * t + rb_, :]
                        src = OSB[t][:, ra_ * W_SP:rb_ * W_SP].rearrange(
                            "p (r j) -> p r j", j=W_SP)
                        eng.dma_start(dst, src)

                    def epi_even(ps, on_scalar):
                        t, u = divmod(c0, NCHUNK)
                        ov = OSB[t][:, u * 4 * W_SP:(u + 1) * 4 * W_SP]
                        if on_scalar:
                            nc.scalar.activation(
                                ov, ps[:, :], ident,
                                bias=gb[:, 2 * p:2 * p + 1],
                                scale=gs[:, 2 * p:2 * p + 1])
                        else:
                            nc.vector.tensor_scalar(
                                ov, ps[:, :], gs[:, 2 * p:2 * p + 1],
                                gb[:, 2 * p:2 * p + 1], mult, add)

                    def epi_odd(ps):
                        # psO: partitions 64:128 hold sample A, 0:64 B
                        t, u = divmod(c1, NCHUNK)
                        ov = OSB[t][:, u * 4 * W_SP:(u + 1) * 4 * W_SP]
                        nc.vector.tensor_scalar(
                            ov[0:64], ps[64:128],
                            gs[64:128, 2 * p + 1:2 * p + 2],
                            gb[64:128, 2 * p + 1:2 * p + 2], mult, add)
                        nc.scalar.activation(
                            ov[64:128], ps[0:64], ident,
                            bias=gb[0:64, 2 * p + 1:2 * p + 2],
                            scale=gs[0:64, 2 * p + 1:2 * p + 2])

                    if last:
                        # tail: epilogue op time scales with free size,
                        # and flush row-ranges are column ranges of the
                        # chunk -- quarter-granularity epilogues (right
                        # rows first) let the final 2-row flush dispatch
                        # ~0.45us after the last matmul instead of ~1.4us
                        t3 = NT - 1
                        HW2 = 2 * W_SP
                        u6 = c1 % NCHUNK
                        u5 = c0 % NCHUNK
                        ovO = OSB[t3][:, u6 * 4 * W_SP:(u6 + 1) * 4 * W_SP]
                        ovE = OSB[t3][:, u5 * 4 * W_SP:(u5 + 1) * 4 * W_SP]
                        for lo, hi, r0_, r1_, eng in (
                                (HW2, 2 * HW2, 26, 28, nc.sync),
                                (0, HW2, 24, 26, nc.scalar)):
                            nc.vector.tensor_scalar(
                                ovO[0:64, lo:hi], psO[64:128, lo:hi],
                                gs[64:128, 2 * p + 1:2 * p + 2],
                                gb[64:128, 2 * p + 1:2 * p + 2], mult, add)
                            nc.scalar.activation(
                                ovO[64:128, lo:hi], psO[0:64, lo:hi], ident,
                                bias=gb[0:64, 2 * p + 1:2 * p + 2],
                                scale=gs[0:64, 2 * p + 1:2 * p + 2])
                            flush(t3, r0_, r1_, eng)
                        # even chunk: column halves across both engines
                        nc.vector.tensor_scalar(
                            ovE[:, HW2:2 * HW2], psE[:, HW2:2 * HW2],
                            gs[:, 2 * p:2 * p + 1],
                            gb[:, 2 * p:2 * p + 1], mult, add)
                        nc.scalar.activation(
                            ovE[:, 0:HW2], psE[:, 0:HW2], ident,
                            bias=gb[:, 2 * p:2 * p + 1],
                            scale=gs[:, 2 * p:2 * p + 1])
                        flush(t3, 20, 24, nc.sync)
                        continue
                    # ---- epilogue: (psum * g) + bias*g, compact pads
                    #      away; work split between VectorE and ScalarE;
                    #      flushes ride the sync queue so a waiting flush
                    #      never delays PSUM eviction ----
                    epi_even(psE, on_scalar=(k % 2 == 0))
                    epi_odd(psO)
                    for c in (c0, c1):
                        t, u = divmod(c, NCHUNK)
                        if u == 3:
                            # the very first big store would drain inside
                            # the load ramp's tightest window (T1-T3 +
                            # next pair's tiles share the SDMA engines);
                            # defer it one iteration
                            if not (p == 0 and t == 0):
                                flush(t, 0, 16, nc.sync)
                        elif u == 4 and p == 0 and t == 0:
                            flush(t, 0, 16, nc.sync)
                        elif u == 4 and p == 1 and t == NT - 1:
                            # pre-drain rows 16-20 so the final u5 flush
                            # is only 4 rows behind the last epilogue
                            flush(t, 16, 20, nc.sync)
                        elif u == 5:
                            flush(t, 16, 24, nc.sync)
                        elif u == 6:
                            flush(t, 24, 28, nc.sync)

    nc.compile()
    return nc


def _prep_inputs(x, W, bias, alpha, label):
    label = np.asarray(label).astype(np.int64)
    af = np.asarray(alpha, np.float32)
    g = 1.0 / (1.0 + np.exp(-af[label]))          # [B, COUT] f32
    gbv = g * np.asarray(bias, np.float32)[None, :]

    # weights: [128, 9*64] bf16; rows 0:64 and 64:128 both = W[cout,cin,dy,dx]
    # arranged as w64[cin, tap*64 + cout]
    wf = np.asarray(W, np.float32)                # [COUT, CIN, 3, 3]
    w64 = np.transpose(wf, (1, 2, 3, 0)).reshape(CIN, 9 * COUT)
    w128 = np.concatenate([w64, w64], axis=0).astype(ml_dtypes.bfloat16)

    # x padded to [114, 114] with zero borders so conv taps never leave
    # their slot and tile loads are fully contiguous
    xb = np.zeros((B, CIN, H + 2, SLOT), dtype=ml_dtypes.bfloat16)
    xb[:, :, 1:H + 1, 1:W_SP + 1] = np.asarray(x, np.float32).astype(
        ml_dtypes.bfloat16)

    in_maps = []
    for core in range(N_CORES):
        s = core * B_LOC
        gsc = np.zeros((128, 4), np.float32)
        gbc = np.zeros((128, 4), np.float32)
        for p in range(2):
            a, b = s + 2 * p, s + 2 * p + 1
            gsc[0:64, 2 * p] = g[a]
            gsc[64:128, 2 * p] = g[b]
            gsc[0:64, 2 * p + 1] = g[b]      # swapped parity
            gsc[64:128, 2 * p + 1] = g[a]
            gbc[0:64, 2 * p] = gbv[a]
            gbc[64:128, 2 * p] = gbv[b]
            gbc[0:64, 2 * p + 1] = gbv[b]
            gbc[64:128, 2 * p + 1] = gbv[a]
        in_maps.append({
            "x": np.ascontiguousarray(
                xb[s:s + B_LOC].reshape(B_LOC * CIN, H + 2, SLOT)),
            "w": w128,
            "gs": gsc,
            "gb": gbc,
        })
    return in_maps


def kernel(x, W, bias, alpha, label):
    global _cached
    from concourse.bass_utils import run_bass_kernel_spmd

    if _cached is None:
        _cached = _build()
    nc = _cached
    in_maps = _prep_inputs(x, W, bias, alpha, label)
    res = run_bass_kernel_spmd(nc, in_maps, core_ids=list(range(N_CORES)))
    out = np.concatenate(
        [np.asarray(res.results[i]["out"], np.float32).reshape(
            B_LOC, COUT, H, W_SP) for i in range(N_CORES)], axis=0)
    return out


# revision 31
# speedup vs baseline: 1.1772x; 1.1772x over previous
"""AlphaWeightedConv2d Trainium2 kernel.

Reference computation (B=32, CIN=COUT=64, H=W=112, K=3, pad=1):
    g = sigmoid(alpha[label])                     # [B, COUT]
    y = conv2d(x, W) * g[:,:,None,None] + (bias * g)[:,:,None,None]

Strategy: data-parallel over batch across 8 NeuronCores (4 samples/core).
Per core the conv is expressed as 9 shifted K=64 matmuls per output chunk
(CIN on partitions) over a host-padded image layout ([114,114] with zero
borders), so every conv tap is a plain (slot, column) window into one
contiguous SBUF tile and every DMA load is a single contiguous span per
partition (fat descriptors -> near line-rate HBM reads).  Two samples
ride in the two 64-partition halves of each tile; even/odd output chunks
map onto the four 64x64 quadrants of the PE array (4 concurrent matmul
streams, separate PSUM banks).  The sigmoid gate is computed on host
([32,64] - negligible) and applied by the DVE/ACT epilogue as a
per-partition scale+bias while evacuating PSUM.  Warm-up matmuls on a
memset tile release the PE HAM clock gate (1.2 -> 2.4 GHz) while the
first loads are in flight.  x is cast to bf16 on host (tolerance allows
it; halves input HBM traffic); output is written bf16 and upcast to f32
on host.
"""

import numpy as np
import ml_dtypes

B, CIN, COUT, H, W_SP = 32, 64, 64, 112, 112
N_CORES = 8
B_LOC = B // N_CORES          # 4 samples per core
SLOT = 114                    # padded row width (1 + 112 + 1)
NSLOT = 30                    # row slots per tile (28 rows + halo pair)
TW = NSLOT * SLOT             # 3420 tile width
R = 28                        # image rows per tile
NT = 4                        # row tiles per sample (4*28 = 112)
NCHUNK = (H // (NT * 4))      # 7 chunks per tile
TAPS = [(dy, dx) for dy in range(3) for dx in range(3)]

_cached = None


def _build():
    from concourse import bacc, tile, mybir

    bf16 = mybir.dt.bfloat16
    f32 = mybir.dt.float32
    mult = mybir.AluOpType.mult
    add = mybir.AluOpType.add
    ident = mybir.ActivationFunctionType.Identity

    nc = bacc.Bacc("TRN2", target_bir_lowering=False, debug=False,
                   num_devices=N_CORES)
    x_ext = nc.dram_tensor("x", [B_LOC * CIN, H + 2, SLOT], bf16,
                           kind="ExternalInput")
    w_ext = nc.dram_tensor("w", [128, 9 * 64], bf16, kind="ExternalInput")
    gs_ext = nc.dram_tensor("gs", [128, 4], f32, kind="ExternalInput")
    gb_ext = nc.dram_tensor("gb", [128, 4], f32, kind="ExternalInput")
    out_ext = nc.dram_tensor("out", [B_LOC * COUT, H, W_SP], bf16,
                             kind="ExternalOutput")

    with tile.TileContext(nc) as tc:
        with (
            tc.tile_pool(name="wpool", bufs=1) as wpool,
            tc.tile_pool(name="xpool", bufs=8) as xpool,
            tc.tile_pool(name="opool", bufs=8) as opool,
            tc.tile_pool(name="pspool", bufs=8, space="PSUM") as pspool,
        ):
            w = wpool.tile([128, 9 * 64], bf16)
            gs = wpool.tile([128, 4], f32)
            gb = wpool.tile([128, 4], f32)

            def load_tile(p, t, split=False):
                # tile t holds padded rows [28t, 28t+30): one contiguous
                # 6840B span per partition -> a single fat DMA descriptor
                xt = xpool.tile([128, TW], bf16, tag="xtile", name=f"x{p}{t}")
                dv = xt[:, :].rearrange("p (s j) -> p s j", j=SLOT)
                sv = x_ext.ap()[p * 128:(p + 1) * 128,
                                R * t:R * t + NSLOT, :]
                if split:
                    # first tile in k-iteration-sized pieces: the early
                    # stream is load-rate-limited (per-engine SDMA cap),
                    # so finer splits only starve later iterations
                    nc.sync.dma_start(dv[:, 0:10], sv[:, 0:10])
                    nc.sync.dma_start(dv[:, 10:20], sv[:, 10:20])
                    nc.sync.dma_start(dv[:, 20:NSLOT], sv[:, 20:NSLOT])
                else:
                    nc.sync.dma_start(dv, sv)
                return xt

            # critical chain first on the sync queue (the scalar queue's
            # HWDGE sits behind a ~1.3us ACT_TABLE_LOAD at startup)
            nc.sync.dma_start(w[:], w_ext.ap()[:])
            T_next = [load_tile(0, 0, split=True)]
            nc.scalar.dma_start(gs[:], gs_ext.ap()[:])
            nc.scalar.dma_start(gb[:], gb_ext.ap()[:])
            T_next += [load_tile(0, t) for t in range(1, NT)]

            # ---- PE warm-up: the HAM clock gate starts at 1.2 GHz and
            # needs ~3.4us of sustained matmul activity to release to
            # 2.4 GHz.  The PE is idle while the first loads are in
            # flight anyway, so burn that window on matmuls over a
            # memset tile (no DMA dependency - starts right after the
            # framework preamble). ----
            wu = wpool.tile([128, 512], bf16)
            nc.gpsimd.memset(wu[:], 0.0)
            psw = pspool.tile([128, 4 * W_SP], f32, tag="ps")
            for _ in range(10):
                nc.tensor.matmul(psw[:, :], wu[:, 0:128], wu[:, 0:448],
                                 start=True, stop=True)

            for p in range(2):  # sample pairs (2p, 2p+1)
                T = T_next
                T_next = []
                OSB = [opool.tile([128, R * W_SP], bf16, tag="osb",
                                  name=f"osb{p}{t}")
                       for t in range(NT)]

                # ---- 28 chunks in pairs: even->psE quadrants (0,0)/(64,64),
                #      odd->psO quadrants (0,64)/(64,0) ----
                for k in range(NT * NCHUNK // 2):
                    # spread the next pair's tile loads across this pair's
                    # compute, ahead of this k's flushes in queue order
                    if p == 0 and k in (1, 4, 7, 10):
                        T_next.append(load_tile(1, len(T_next)))
                    c0, c1 = 2 * k, 2 * k + 1
                    last = (p == 1 and k == NT * NCHUNK // 2 - 1)
                    if last:
                        # scheduler breaks ties by PSUM allocation order;
                        # the odd chunk gates the final flush, so give it
                        # priority in the tail
                        psO = pspool.tile([128, 4 * W_SP], f32, tag="ps",
                                          name="psO_l")
                        psE = pspool.tile([128, 4 * W_SP], f32, tag="ps",
                                          name="psE_l")
                    else:
                        psE = pspool.tile([128, 4 * W_SP], f32, tag="ps")
                        psO = pspool.tile([128, 4 * W_SP], f32, tag="ps")
                    for i, (dy, dx) in enumerate(TAPS):
                        st, sp = i == 0, i == 8
                        for c, ps, swap in ((c0, psE, False), (c1, psO, True)):
                            t, u = divmod(c, NCHUNK)
                            rv = T[t][:, :].rearrange(
                                "p (s j) -> p s j", j=SLOT)
                            ra = rv[0:64, 4 * u + dy:4 * u + dy + 4,
                                    dx:dx + 112]
                            rb = rv[64:128, 4 * u + dy:4 * u + dy + 4,
                                    dx:dx + 112]
                            aslice = ps[64:128] if swap else ps[0:64]
                            bslice = ps[0:64] if swap else ps[64:128]
                            nc.tensor.matmul(
                                aslice.rearrange("p (r j) -> p r j", j=W_SP),
                                w[0:64, i * 64:(i + 1) * 64],
                                ra, start=st, stop=sp)
                            nc.tensor.matmul(
                                bslice.rearrange("p (r j) -> p r j", j=W_SP),
                                w[64:128, i * 64:(i + 1) * 64],
                                rb, start=st, stop=sp)
                    def flush(t, ra_, rb_, eng):
                        dst = out_ext.ap()[p * 128:(p + 1) * 128,
                                           R * t + ra_:R * t + rb_, :]
                        src = OSB[t][:, ra_ * W_SP:rb_ * W_SP].rearrange(
                            "p (r j) -> p r j", j=W_SP)
                        eng.dma_start(dst, src)

                    def epi_even(ps, on_scalar):
                        t, u = divmod(c0, NCHUNK)
                        ov = OSB[t][:, u * 4 * W_SP:(u + 1) * 4 * W_SP]
                        if on_scalar:
                            nc.scalar.activation(
                                ov, ps[:, :], ident,
                                bias=gb[:, 2 * p:2 * p + 1],
                                scale=gs[:, 2 * p:2 * p + 1])
                        else:
                            nc.vector.tensor_scalar(
                                ov, ps[:, :], gs[:, 2 * p:2 * p + 1],
                                gb[:, 2 * p:2 * p + 1], mult, add)

                    def epi_odd(ps):
                        # psO: partitions 64:128 hold sample A, 0:64 B
                        t, u = divmod(c1, NCHUNK)
                        ov = OSB[t][:, u * 4 * W_SP:(u + 1) * 4 * W_SP]
                        nc.vector.tensor_scalar(
                            ov[0:64], ps[64:128],
                            gs[64:128, 2 * p + 1:2 * p + 2],
                            gb[64:128, 2 * p + 1:2 * p + 2], mult, add)
                        nc.scalar.activation(
                            ov[64:128], ps[0:64], ident,
                            bias=gb[0:64, 2 * p + 1:2 * p + 2],
                            scale=gs[0:64, 2 * p + 1:2 * p + 2])

                    if last:
                        # tail: epilogue op time scales with free size,
                        # and flush row-ranges are column ranges of the
                        # chunk -- quarter-granularity epilogues (right
                        # rows first) let the final 2-row flush dispatch
                        # ~0.45us after the last matmul instead of ~1.4us
                        t3 = NT - 1
                        HW2 = 2 * W_SP
                        u6 = c1 % NCHUNK
                        u5 = c0 % NCHUNK
                        ovO = OSB[t3][:, u6 * 4 * W_SP:(u6 + 1) * 4 * W_SP]
                        ovE = OSB[t3][:, u5 * 4 * W_SP:(u5 + 1) * 4 * W_SP]
                        for lo, hi, r0_, r1_, eng in (
                                (HW2, 2 * HW2, 26, 28, nc.sync),
                                (0, HW2, 24, 26, nc.scalar)):
                            nc.vector.tensor_scalar(
                                ovO[0:64, lo:hi], psO[64:128, lo:hi],
                                gs[64:128, 2 * p + 1:2 * p + 2],
                                gb[64:128, 2 * p + 1:2 * p + 2], mult, add)
                            nc.scalar.activation(
                                ovO[64:128, lo:hi], psO[0:64, lo:hi], ident,
                                bias=gb[0:64, 2 * p + 1:2 * p + 2],
                                scale=gs[0:64, 2 * p + 1:2 * p + 2])
                            flush(t3, r0_, r1_, eng)
                        # even chunk: column halves across both engines
                        nc.vector.tensor_scalar(
                            ovE[:, HW2:2 * HW2], psE[:, HW2:2 * HW2],
                            gs[:, 2 * p:2 * p + 1],
                            gb[:, 2 * p:2 * p + 1], mult, add)
                        nc.scalar.activation(
                            ovE[:, 0:HW2], psE[:, 0:HW2], ident,
                            bias=gb[:, 2 * p:2 * p + 1],
                            scale=gs[:, 2 * p:2 * p + 1])
                        flush(t3, 20, 24, nc.sync)
                        continue
                    # ---- epilogue: (psum * g) + bias*g, compact pads
                    #      away; work split between VectorE and ScalarE;
                    #      flushes ride the sync queue so a waiting flush
                    #      never delays PSUM eviction ----
                    epi_even(psE, on_scalar=(k % 2 == 0))
                    epi_odd(psO)
                    for c in (c0, c1):
                        t, u = divmod(c, NCHUNK)
                        if u == 3:
                            flush(t, 0, 16, nc.sync)
                        elif u == 4 and p == 1 and t == NT - 1:
                            # pre-drain rows 16-20 so the final u5 flush
                            # is only 4 rows behind the last epilogue
                            flush(t, 16, 20, nc.sync)
                        elif u == 5:
                            flush(t, 16, 24, nc.sync)
                        elif u == 6:
                            flush(t, 24, 28, nc.sync)

    nc.compile()
    return nc


def _prep_inputs(x, W, bias, alpha, label):
    label = np.asarray(label).astype(np.int64)
    af = np.asarray(alpha, np.float32)
    g = 1.0 / (1.0 + np.exp(-af[label]))          # [B, COUT] f32
    gbv = g * np.asarray(bias, np.float32)[None, :]

    # weights: [128, 9*64] bf16; rows 0:64 and 64:128 both = W[cout,cin,dy,dx]
    # arranged as w64[cin, tap*64 + cout]
    wf = np.asarray(W, np.float32)                # [COUT, CIN, 3, 3]
    w64 = np.transpose(wf, (1, 2, 3, 0)).reshape(CIN, 9 * COUT)
    w128 = np.concatenate([w64, w64], axis=0).astype(ml_dtypes.bfloat16)

    # x padded to [114, 114] with zero borders so conv taps never leave
    # their slot and tile loads are fully contiguous
    xb = np.zeros((B, CIN, H + 2, SLOT), dtype=ml_dtypes.bfloat16)
    xb[:, :, 1:H + 1, 1:W_SP + 1] = np.asarray(x, np.float32).astype(
        ml_dtypes.bfloat16)

    in_maps = []
    for core in range(N_CORES):
        s = core * B_LOC
        gsc = np.zeros((128, 4), np.float32)
        gbc = np.zeros((128, 4), np.float32)
        for p in range(2):
            a, b = s + 2 * p, s + 2 * p + 1
            gsc[0:64, 2 * p] = g[a]
            gsc[64:128, 2 * p] = g[b]
            gsc[0:64, 2 * p + 1] = g[b]      # swapped parity
            gsc[64:128, 2 * p + 1] = g[a]
            gbc[0:64, 2 * p] = gbv[a]
            gbc[64:128, 2 * p] = gbv[b]
            gbc[0:64, 2 * p + 1] = gbv[b]
            gbc[64:128, 2 * p + 1] = gbv[a]
        in_maps.append({
            "x": np.ascontiguousarray(
                xb[s:s + B_LOC].reshape(B_LOC * CIN, H + 2, SLOT)),
            "w": w128,
            "gs": gsc,
            "gb": gbc,
        })
    return in_maps


def kernel(x, W, bias, alpha, label):
    global _cached
    from concourse.bass_utils import run_bass_kernel_spmd

    if _cached is None:
        _cached = _build()
    nc = _cached
    in_maps = _prep_inputs(x, W, bias, alpha, label)
    res = run_bass_kernel_spmd(nc, in_maps, core_ids=list(range(N_CORES)))
    out = np.concatenate(
        [np.asarray(res.results[i]["out"], np.float32).reshape(
            B_LOC, COUT, H, W_SP) for i in range(N_CORES)], axis=0)
    return out
